# revision 51
# baseline (speedup 1.0000x reference)
"""BlockGrouper (MoE routing dispatch) Trainium2 kernel — raw bass.

Semantics (from the reference): each token n in sample b belongs to group
g = argmax(block_onehot[b, n]); its slot within the group is its rank
among same-group tokens in token order.  With the balanced one-hot
routing, the output [B, G, cap, D] is a pure row-permutation of
x [B, N, D].

Sharding: data-parallel over B across the 8 NeuronCores (one sample per
core); each core moves 16 MiB in + 16 MiB out.

The data phase is a pure-write dynamic indirect scatter (InstDMACopy,
cce_op=bypass): 64 calls x 128 rows of 2 KiB.  HW-verified ucode
constraints (micro-benched, see probe.py / probe2.py):
  - exactly ONE offset per partition per call, offsets [128, 1] int32,
    payload 2D [128, elem] (elem length is flexible but contiguous per
    partition); multi-column offset APs degenerate into a broken
    single-partition walk (indices read sequentially from partition 0,
    overlapping copies, address wrap mod 2^18 B) — unusable;
  - the `queue` attribute is ignored: all generic InstDMACopy descs go
    to SWDGE queue 0 and their desc-gen serializes on the Pool engine
    (~1114 ns ucode + ~300 ns sequencer gap per call).  Only the *Ant
    instructions honor queue_num (their desc-gen runs on parallel Q7 cpu
    pairs), but dma_scatter_add is CCE read-modify-write and drains at
    only ~26 GB/s/queue — measured strictly worse in the mix;
  - coef is in elements of the out dtype (512 here).
The scatter phase is therefore Pool-desc-gen-bound at 64 x ~1.42 us =
91 us; with the ~16 us head (oh load + index stream) and ~3 us of tail
this lands at ~110-112 us measured (best 109.7; device-state dependent
— the same binary measures ~+18% in degraded clock windows, so always
re-run before trusting a regression).  HBM-roofline for the 32.25
MiB/core at the measured ~390-420 GB/s peak would be ~95 us.

Per-core program (N=8192, G=16, D=512, cap=512, P=128, C=64; token n
lives at partition p = n // 64, column c = n % 64):
  1. Index pipeline: tot[p, g] = per-partition group counts (contiguous
     pre-add + strided DVE reduce); PE computes the carry a_ps[p, g] =
     (# tokens of g before partition p) + g*cap - 1 via one
     strict-upper-triangular-ones matmul plus a const-row matmul; then a
     STREAMING per-column recurrence (4 tiny contiguous [128, 16] DVE
     ops per column: mult by the running row-sum R, R += oh_c, reduce
     over g, cast) emits dest_i columns at ~0.6 us/col — ahead of the
     Pool's 1.42 us/col consumption — with per-column s_dve increments
     so the first scatter fires at ~16 us.  Every same-engine RAW/WAR
     pair is kept >= 2 instructions apart (DVE does not interlock close
     hazards; violating this corrupts results on HW).
  2. Data path: oh rides the SP HWDGE ring alone (it gates the index
     pipeline); the ACT ring leads with the constants then starts x
     immediately.  x-chunk loads (p-major, contiguous per partition,
     sizes ramping [2,2,4,8,8...] so early scatter columns unblock
     first; consolidating to fewer/bigger chunks measurably starves the
     early columns) split across both rings with per-ring FIFO
     completion sems, then 64 single-column indirect scatter-writes on
     SWDGE queue 0.  A dummy scatter at t=0 warms the dynamic-DMA path
     off the critical path.
"""


import numpy as np

B, N, G, D = 8, 8192, 16, 512
CAP = N // G
P = 128
C = N // P
NCORES = 8
# x-load chunks in token-columns: small first chunks so the first scatter
# columns unblock early, bigger later ones to amortize DMA count
CHUNK_COLS = [2, 2, 4, 8, 8, 8, 8, 8, 8, 8]
CHUNKS = []
_c = 0
for _w in CHUNK_COLS:
    CHUNKS.append((_c, _c + _w))
    _c += _w
assert _c == C
NCHUNK = len(CHUNKS)

_cached = None


def _indirect_scatter_write(nc, out_ap, offset_ap, in_ap, queue_name):
    """nc.gpsimd.indirect_dma_start(out, offset(axis 0), in_, bypass) with a
    parameterized SWDGE queue name (the stock method pins qPoolDynamic)."""
    import concourse.mybir as mybir

    eng = nc.gpsimd
    out_l = eng.lower_ap_dma(out_ap, for_indirect_dma=True)
    in_l = eng.lower_ap_dma(in_ap, for_indirect_dma=True)
    assert len(out_l) == 1 and len(in_l) == 1
    off_l = eng.lower_ap_dma(offset_ap)
    assert len(off_l) == 1
    in_l.append(off_l[0])

    coef = out_ap.shape[1]  # elements per row of the indirect'd axis 0
    out_l[0].dynamic_ap_info = mybir.DynamicAccessPatternInfo(
        c=0,
        actual_ap=in_ap.ap,
        indirect_dim_max_index=out_ap.shape[0],
        offset_expr=[
            mybir.DynamicAccessPatternOffsetExpr(
                coef=coef,
                aff_expr=mybir.DynamicAccessPatternOffsetExprAffExpr(
                    kind="IndirectArgId", arg_id=1
                ),
            )
        ],
    )
    return eng.add_instruction(
        mybir.InstDMACopy(
            name=nc.get_next_instruction_name(),
            queue=queue_name,
            mode="Copy",
            ins=in_l,
            outs=out_l,
            oob_is_err=False,
            cce_op=mybir.AluOpType.bypass,
        )
    )


def _build():
    import contextlib

    import concourse.bass as bass
    import concourse.bacc as bacc
    import concourse.mybir as mybir

    f32 = mybir.dt.float32
    i32 = mybir.dt.int32
    i16 = mybir.dt.int16

    nc = bacc.Bacc("TRN2", target_bir_lowering=False, debug=False,
                   num_devices=NCORES, num_swdge_queues=1,
                   detect_race_conditions=False)
    x_d = nc.dram_tensor("x", [N, D], f32, kind="ExternalInput")
    oh_d = nc.dram_tensor("oh", [N, G], f32, kind="ExternalInput")
    cst_big_d = nc.dram_tensor("cst_big", [P, P], f32,
                               kind="ExternalInput")
    cst_row_d = nc.dram_tensor("cst_row", [1, P + G], f32,
                               kind="ExternalInput")
    out_d = nc.dram_tensor("out", [N, D], f32, kind="ExternalOutput")
    # tiny scratch target for the t=0 dummy scatter that preloads any lazy
    # dynamic-DMA library off the critical path
    dummy_d = nc.dram_tensor("lib_warm", [16, 64], f32, kind="ExternalOutput")

    with (
        nc.sbuf_tensor("cst_big_t", [P, P], f32) as cst_big_t,
        nc.sbuf_tensor("cst_row_t", [1, P + G], f32) as cst_row_t,
        nc.sbuf_tensor("oh_t", [P, C * G], f32) as oh_t,
        nc.sbuf_tensor("tot_t", [P, G], f32) as tot_t,
        nc.sbuf_tensor("scan_t", [P, C * G], f32) as scan_t,
        nc.sbuf_tensor("prod_t", [P, C * G], f32) as prod_t,
        nc.sbuf_tensor("dest_f", [P, C], f32) as dest_f,
        nc.sbuf_tensor("dest_i", [P, C], i32) as dest_i,
        nc.sbuf_tensor("xt", [P, C * D], f32) as xt,
        nc.psum_tensor("a_ps", [P, G], f32) as a_ps,
        contextlib.ExitStack() as stack,
        nc.semaphore("s_const") as s_const,
        nc.semaphore("s_oh") as s_oh,
        nc.semaphore("s_dve") as s_dve,
        nc.semaphore("s_pe") as s_pe,
        nc.semaphore("s_warm") as s_warm,
    ):
        # per-ring x-load sems: chunk k fully arrived iff both rings have
        # delivered their k-th half (each ring is FIFO)
        s_xs = stack.enter_context(nc.semaphore("s_xs"))
        s_xc = stack.enter_context(nc.semaphore("s_xc"))
        s_sq0 = stack.enter_context(nc.semaphore("s_sq0"))
        dummy_idx = stack.enter_context(
            nc.sbuf_tensor("dummy_idx", [P, 1], i32))
        dummy_pay = stack.enter_context(
            nc.sbuf_tensor("dummy_pay", [P, 64], f32))
        su_t = cst_big_t[:, 0:P]
        ones_t = cst_row_t[:, 0:P]
        cst_t = cst_row_t[:, P:P + G]

        # ---------------- plain DMAs ----------------
        # oh gates the whole index pipeline: split it across BOTH HWDGE
        # rings (SP + ACT) so it lands ~1.3us earlier; constants follow on
        # the ACT ring.
        # oh rides the SP ring alone (gates the index pipeline); the ACT
        # ring leads with the tiny constants and then starts on x chunks
        # immediately, so early chunks outrun the scatter's consumption
        nc.sync.dma_start(
            out=oh_t[:],
            in_=oh_d[:].rearrange("(p c) g -> p (c g)", p=P)).then_inc(
            s_oh, 16)
        nc.scalar.dma_start(out=cst_big_t[:], in_=cst_big_d[:]).then_inc(
            s_const, 16)
        nc.scalar.dma_start(out=cst_row_t[:], in_=cst_row_d[:]).then_inc(
            s_const, 16)
        # p-major: the scatter for column c carries x rows for tokens
        # p * 64 + c.  Chunk sizes ramp up (small first chunks) so the
        # first scatter columns unblock as early as possible; halves of
        # each chunk go to the two HWDGE rings.
        x3 = x_d[:].rearrange("(p c) d -> p c d", p=P)
        xto = xt[:].rearrange("p (c d) -> p c d", d=D)
        for k, (c0, c1) in enumerate(CHUNKS):
            h = (c1 - c0) // 2
            nc.sync.dma_start(
                out=xto[:, c0:c0 + h, :],
                in_=x3[:, c0:c0 + h, :]).then_inc(s_xs, 16)
            nc.scalar.dma_start(
                out=xto[:, c0 + h:c1, :],
                in_=x3[:, c0 + h:c1, :]).then_inc(s_xc, 16)

        # ---------------- DVE ----------------
        # tot[p, g] = number of group-g tokens in partition p; the PE turns
        # it into the carry a_ps[p, g] = (tokens of g before partition p)
        # + g*cap - 1.  The scans then start from that carry directly, so
        # scan_g[p, c] == dest for group-g tokens; prod*reduce collapses
        # over g.  Same-engine RAW pairs are kept >= 2 instructions apart
        # (DVE does not interlock close RAW hazards).
        nc.vector.wait_ge(s_oh, 16)
        # tot[p, g] = sum_c oh[p, c, g]: contiguous pre-add of the column
        # halves, then a strided reduce over the remaining 32 columns
        nc.vector.tensor_tensor(
            out=scan_t[:, 0:C * G // 2], in0=oh_t[:, 0:C * G // 2],
            in1=oh_t[:, C * G // 2:C * G], op=mybir.AluOpType.add)
        nc.vector.tensor_reduce(
            out=tot_t[:],
            in_=scan_t[:, 0:C * G // 2].rearrange("p (c g) -> p g c", g=G),
            axis=mybir.AxisListType.X,
            op=mybir.AluOpType.add).then_inc(s_dve, 1)

        # Streaming per-column recurrence: with R[p, g] = a_ps[p, g] + 1
        # + sum_{c'<c} oh[p, c', g], the destination of token (p, c) is
        # dest[p, c] = sum_g oh[p, c, g] * R[p, g] (oh is one-hot over g,
        # and oh_c[g*] = 1 folds the token's own +1 into R's +1).  Four
        # tiny contiguous [128, 16] ops per column stream dest columns out
        # faster than the Pool consumes them (1.42us/col), so the first
        # scatter fires ~3us earlier than a bulk scan pipeline.  Emission
        # order keeps every same-engine RAW/WAR pair >= 2 instructions
        # apart (DVE does not interlock close hazards).
        nc.vector.wait_ge(s_pe, 1)
        r_t = scan_t[:, 0:G]
        m_t = prod_t
        nc.vector.tensor_scalar(out=r_t, in0=a_ps[:], scalar1=1.0,
                                scalar2=None, op0=mybir.AluOpType.add)

        def mult_c(c):
            nc.vector.tensor_tensor(
                out=m_t[:, c * G:(c + 1) * G],
                in0=oh_t[:, c * G:(c + 1) * G], in1=r_t,
                op=mybir.AluOpType.mult)

        def racc_c(c):
            nc.vector.tensor_tensor(
                out=r_t, in0=r_t, in1=oh_t[:, c * G:(c + 1) * G],
                op=mybir.AluOpType.add)

        def reduce_c(c):
            nc.vector.tensor_reduce(
                out=dest_f[:, c:c + 1],
                in_=m_t[:, c * G:(c + 1) * G].rearrange(
                    "p (c g) -> p c g", g=G),
                axis=mybir.AxisListType.X,
                op=mybir.AluOpType.add)

        def cast_c(c):
            # dest_i[:, c] ready => s_dve hits 2 + c
            nc.vector.tensor_copy(
                out=dest_i[:, c:c + 1],
                in_=dest_f[:, c:c + 1]).then_inc(s_dve, 1)

        def spacer(i):
            # harmless filler op to keep hazard distances >= 2 during boot
            nc.vector.tensor_copy(out=m_t[:, C * G - 1 - i:C * G - i],
                                  in_=oh_t[:, 0:1])

        # steady schedule: mult_c, cast_{c-1}, racc_c, reduce_c — every
        # RAW *and* WAR pair on r_t/m_t/dest_f lands >= 2 instructions
        # apart (the spacers cover the boot distances)
        spacer(0)
        mult_c(0)
        spacer(1)
        racc_c(0)
        reduce_c(0)
        for c in range(1, C):
            mult_c(c)
            cast_c(c - 1)
            racc_c(c)
            reduce_c(c)
        spacer(2)
        cast_c(C - 1)

        # ---------------- PE ----------------
        nc.tensor.wait_ge(s_const, 32)
        nc.tensor.wait_ge(s_dve, 1)
        nc.tensor.matmul(out=a_ps[:], lhsT=su_t, rhs=tot_t[:],
                         start=True, stop=False)
        nc.tensor.matmul(out=a_ps[:], lhsT=ones_t, rhs=cst_t,
                         start=False, stop=True).then_inc(s_pe, 1)

        # ---------------- Pool: indirect scatter-writes ----------------
        qname = ["qPoolDynamic", "qPoolDynamic1", "qPoolDynamic2",
                 "qPoolDynamic3"]
        # dummy scatter at t=0 warms the dynamic-DMA path.  The ucode only
        # supports one offset per partition and a 2D [128, D] payload per
        # call, so the main loop is one call per token column: 64 calls x
        # 128 rows of 2 KiB (~1.1 us of Pool desc-gen each, measured).
        # v4: no dma_scatter_add columns at all — the Ant calls cost
        # ~5.7 us each of serial Pool time plus a ~12 us LOAD_LIB stall in
        # front of the generic calls, and their CCE RMW re-reads the
        # output (4 MiB extra HBM traffic).  All-generic is both cheaper
        # on the Pool queue and lighter on the bus.
        nc.gpsimd.memset(dummy_idx[:], 0).then_inc(s_warm, 1)
        nc.gpsimd.memset(dummy_pay[:], 0).then_inc(s_warm, 1)
        nc.gpsimd.wait_ge(s_warm, 2)
        _indirect_scatter_write(
            nc, dummy_d[:], dummy_idx[:], dummy_pay[:],
            qname[0]).then_inc(s_sq0, 16)

        chunk_start = {c0: k for k, (c0, c1) in enumerate(CHUNKS)}

        # dest_i column c is covered by s_dve >= 2 + c (per-column
        # casts); waits are per-column early on and coarsen once the
        # index stream is ahead; each coarse wait covers every column up
        # to the next milestone.
        for c in range(C):
            if c in chunk_start:
                k = chunk_start[c]
                nc.gpsimd.wait_ge(s_xs, 16 * (k + 1))
                nc.gpsimd.wait_ge(s_xc, 16 * (k + 1))
            if c < 6:
                nc.gpsimd.wait_ge(s_dve, 2 + c)
            elif c % 8 == 6:
                nc.gpsimd.wait_ge(s_dve, 2 + min(c + 7, C - 1))
            _indirect_scatter_write(
                nc, out_d[:], dest_i[:, c:c + 1],
                xt[:, c * D:(c + 1) * D],
                qname[0]).then_inc(s_sq0, 16)
        nc.gpsimd.wait_ge(s_sq0, 16 * (1 + C))

    nc.compile()
    return nc


def _get_nc():
    global _cached
    if _cached is None:
        _cached = _build()
    return _cached


def _constants():
    cst_big = np.ascontiguousarray(
        np.triu(np.ones((P, P), np.float32), k=1))
    ones_r = np.ones((1, P), np.float32)
    cst = (np.arange(G, dtype=np.float32) * CAP - 1.0).reshape(1, G)
    cst_row = np.concatenate([ones_r, cst], axis=1)
    return cst_big, cst_row


def kernel(x, block_onehot, capacity):
    from concourse.bass_utils import run_bass_kernel_spmd

    x = np.ascontiguousarray(np.asarray(x, dtype=np.float32))
    oh = np.asarray(block_onehot, dtype=np.float32)
    if oh.ndim == 2:
        oh = np.broadcast_to(oh[None], (B,) + oh.shape)
    oh = np.ascontiguousarray(oh)
    assert x.shape == (B, N, D), x.shape
    assert oh.shape == (B, N, G), oh.shape
    assert int(capacity) == CAP, capacity
    nc = _get_nc()
    cst_big, cst_row = _constants()
    in_maps = [
        {"x": x[b], "oh": oh[b], "cst_big": cst_big, "cst_row": cst_row}
        for b in range(B)
    ]
    res = run_bass_kernel_spmd(nc, in_maps, core_ids=list(range(NCORES)))
    return np.stack([res.results[b]["out"].reshape(G, CAP, D)
                     for b in range(B)])



# revision 52
# speedup vs baseline: 1.1410x; 1.1410x over previous
"""BlockGrouper (MoE routing dispatch) Trainium2 kernel — raw bass.

Semantics (from the reference): each token n in sample b belongs to group
g = argmax(block_onehot[b, n]); its slot within the group is its rank
among same-group tokens in token order.  With the balanced one-hot
routing, the output [B, G, cap, D] is a pure row-permutation of
x [B, N, D].

Sharding: data-parallel over B across the 8 NeuronCores (one sample per
core); each core moves 16 MiB in + 16 MiB out.

The data phase is a pure-write dynamic indirect scatter (InstDMACopy,
cce_op=bypass): 64 calls x 128 rows of 2 KiB.  HW-verified ucode
constraints (micro-benched, see probe.py / probe2.py):
  - exactly ONE offset per partition per call, offsets [128, 1] int32,
    payload 2D [128, elem] (elem length is flexible but contiguous per
    partition); multi-column offset APs degenerate into a broken
    single-partition walk (indices read sequentially from partition 0,
    overlapping copies, address wrap mod 2^18 B) — unusable;
  - the `queue` attribute is ignored: all generic InstDMACopy descs go
    to SWDGE queue 0 and their desc-gen serializes on the Pool engine
    (~1114 ns ucode + ~300 ns sequencer gap per call).  Only the *Ant
    instructions honor queue_num (their desc-gen runs on parallel Q7 cpu
    pairs), but dma_scatter_add is CCE read-modify-write and drains at
    only ~26 GB/s/queue — measured strictly worse in the mix;
  - coef is in elements of the out dtype (512 here).
The scatter phase is therefore Pool-desc-gen-bound at 64 x ~1.42 us =
91 us; with the ~16 us head (oh load + index stream) and ~3 us of tail
this lands at ~110-112 us measured (best 109.7; device-state dependent
— the same binary measures ~+18% in degraded clock windows, so always
re-run before trusting a regression).  HBM-roofline for the 32.25
MiB/core at the measured ~390-420 GB/s peak would be ~95 us.

Per-core program (N=8192, G=16, D=512, cap=512, P=128, C=64; token n
lives at partition p = n // 64, column c = n % 64):
  1. Index pipeline: tot[p, g] = per-partition group counts (contiguous
     pre-add + strided DVE reduce); PE computes the carry a_ps[p, g] =
     (# tokens of g before partition p) + g*cap - 1 via one
     strict-upper-triangular-ones matmul plus a const-row matmul; then a
     STREAMING per-column recurrence (4 tiny contiguous [128, 16] DVE
     ops per column: mult by the running row-sum R, R += oh_c, reduce
     over g, cast) emits dest_i columns at ~0.6 us/col — ahead of the
     Pool's 1.42 us/col consumption — with per-column s_dve increments
     so the first scatter fires at ~16 us.  Every same-engine RAW/WAR
     pair is kept >= 2 instructions apart (DVE does not interlock close
     hazards; violating this corrupts results on HW).
  2. Data path: oh rides the SP HWDGE ring alone (it gates the index
     pipeline); the ACT ring leads with the constants then starts x
     immediately.  x-chunk loads (p-major, contiguous per partition,
     sizes ramping [2,2,4,8,8...] so early scatter columns unblock
     first; consolidating to fewer/bigger chunks measurably starves the
     early columns) split across both rings with per-ring FIFO
     completion sems, then 64 single-column indirect scatter-writes on
     SWDGE queue 0.  A dummy scatter at t=0 warms the dynamic-DMA path
     off the critical path.
"""


import numpy as np

B, N, G, D = 8, 8192, 16, 512
CAP = N // G
P = 128
C = N // P
NCORES = 8
# x-load chunks in token-columns: small first chunks so the first scatter
# columns unblock early, bigger later ones to amortize DMA count
CHUNK_COLS = [2, 2, 4, 8, 8, 8, 8, 8, 8, 8]
CHUNKS = []
_c = 0
for _w in CHUNK_COLS:
    CHUNKS.append((_c, _c + _w))
    _c += _w
assert _c == C
NCHUNK = len(CHUNKS)

_cached = None


def _indirect_scatter_write(nc, out_ap, offset_ap, in_ap, queue_name):
    """nc.gpsimd.indirect_dma_start(out, offset(axis 0), in_, bypass) with a
    parameterized SWDGE queue name (the stock method pins qPoolDynamic)."""
    import concourse.mybir as mybir

    eng = nc.gpsimd
    out_l = eng.lower_ap_dma(out_ap, for_indirect_dma=True)
    in_l = eng.lower_ap_dma(in_ap, for_indirect_dma=True)
    assert len(out_l) == 1 and len(in_l) == 1
    off_l = eng.lower_ap_dma(offset_ap)
    assert len(off_l) == 1
    in_l.append(off_l[0])

    coef = out_ap.shape[1]  # elements per row of the indirect'd axis 0
    out_l[0].dynamic_ap_info = mybir.DynamicAccessPatternInfo(
        c=0,
        actual_ap=in_ap.ap,
        indirect_dim_max_index=out_ap.shape[0],
        offset_expr=[
            mybir.DynamicAccessPatternOffsetExpr(
                coef=coef,
                aff_expr=mybir.DynamicAccessPatternOffsetExprAffExpr(
                    kind="IndirectArgId", arg_id=1
                ),
            )
        ],
    )
    return eng.add_instruction(
        mybir.InstDMACopy(
            name=nc.get_next_instruction_name(),
            queue=queue_name,
            mode="Copy",
            ins=in_l,
            outs=out_l,
            oob_is_err=False,
            cce_op=mybir.AluOpType.bypass,
        )
    )


def _build():
    import contextlib

    import concourse.bass as bass
    import concourse.bacc as bacc
    import concourse.mybir as mybir

    f32 = mybir.dt.float32
    i32 = mybir.dt.int32
    i16 = mybir.dt.int16

    nc = bacc.Bacc("TRN2", target_bir_lowering=False, debug=False,
                   num_devices=NCORES, num_swdge_queues=4,
                   detect_race_conditions=False)
    x_d = nc.dram_tensor("x", [N, D], f32, kind="ExternalInput")
    oh_d = nc.dram_tensor("oh", [N, G], f32, kind="ExternalInput")
    cst_big_d = nc.dram_tensor("cst_big", [P, P], f32,
                               kind="ExternalInput")
    cst_row_d = nc.dram_tensor("cst_row", [1, P + G], f32,
                               kind="ExternalInput")
    out_d = nc.dram_tensor("out", [N, D], f32, kind="ExternalOutput")
    # tiny scratch target for the t=0 dummy scatter that preloads any lazy
    # dynamic-DMA library off the critical path
    dummy_d = nc.dram_tensor("lib_warm", [16, 64], f32, kind="ExternalOutput")

    with (
        nc.sbuf_tensor("cst_big_t", [P, P], f32) as cst_big_t,
        nc.sbuf_tensor("cst_row_t", [1, P + G], f32) as cst_row_t,
        nc.sbuf_tensor("oh_t", [P, C * G], f32) as oh_t,
        nc.sbuf_tensor("tot_t", [P, G], f32) as tot_t,
        nc.sbuf_tensor("scan_t", [P, C * G], f32) as scan_t,
        nc.sbuf_tensor("prod_t", [P, C * G], f32) as prod_t,
        nc.sbuf_tensor("dest_f", [P, C], f32) as dest_f,
        nc.sbuf_tensor("dest_i", [P, C], i32) as dest_i,
        nc.sbuf_tensor("xt", [P, C * D], f32) as xt,
        nc.psum_tensor("a_ps", [P, G], f32) as a_ps,
        contextlib.ExitStack() as stack,
        nc.semaphore("s_const") as s_const,
        nc.semaphore("s_oh") as s_oh,
        nc.semaphore("s_dve") as s_dve,
        nc.semaphore("s_pe") as s_pe,
        nc.semaphore("s_warm") as s_warm,
    ):
        # per-ring x-load sems: chunk k fully arrived iff both rings have
        # delivered their k-th half (each ring is FIFO)
        s_xs = stack.enter_context(nc.semaphore("s_xs"))
        s_xc = stack.enter_context(nc.semaphore("s_xc"))
        s_sq0 = stack.enter_context(nc.semaphore("s_sq0"))
        dummy_idx = stack.enter_context(
            nc.sbuf_tensor("dummy_idx", [P, 1], i32))
        dummy_pay = stack.enter_context(
            nc.sbuf_tensor("dummy_pay", [P, 64], f32))
        su_t = cst_big_t[:, 0:P]
        ones_t = cst_row_t[:, 0:P]
        cst_t = cst_row_t[:, P:P + G]

        # ---------------- plain DMAs ----------------
        # oh gates the whole index pipeline: split it across BOTH HWDGE
        # rings (SP + ACT) so it lands ~1.3us earlier; constants follow on
        # the ACT ring.
        # oh rides the SP ring alone (gates the index pipeline); the ACT
        # ring leads with the tiny constants and then starts on x chunks
        # immediately, so early chunks outrun the scatter's consumption
        nc.sync.dma_start(
            out=oh_t[:],
            in_=oh_d[:].rearrange("(p c) g -> p (c g)", p=P)).then_inc(
            s_oh, 16)
        nc.scalar.dma_start(out=cst_big_t[:], in_=cst_big_d[:]).then_inc(
            s_const, 16)
        nc.scalar.dma_start(out=cst_row_t[:], in_=cst_row_d[:]).then_inc(
            s_const, 16)
        # p-major: the scatter for column c carries x rows for tokens
        # p * 64 + c.  Chunk sizes ramp up (small first chunks) so the
        # first scatter columns unblock as early as possible; halves of
        # each chunk go to the two HWDGE rings.
        x3 = x_d[:].rearrange("(p c) d -> p c d", p=P)
        xto = xt[:].rearrange("p (c d) -> p c d", d=D)
        for k, (c0, c1) in enumerate(CHUNKS):
            h = (c1 - c0) // 2
            nc.sync.dma_start(
                out=xto[:, c0:c0 + h, :],
                in_=x3[:, c0:c0 + h, :]).then_inc(s_xs, 16)
            nc.scalar.dma_start(
                out=xto[:, c0 + h:c1, :],
                in_=x3[:, c0 + h:c1, :]).then_inc(s_xc, 16)

        # ---------------- DVE ----------------
        # tot[p, g] = number of group-g tokens in partition p; the PE turns
        # it into the carry a_ps[p, g] = (tokens of g before partition p)
        # + g*cap - 1.  The scans then start from that carry directly, so
        # scan_g[p, c] == dest for group-g tokens; prod*reduce collapses
        # over g.  Same-engine RAW pairs are kept >= 2 instructions apart
        # (DVE does not interlock close RAW hazards).
        nc.vector.wait_ge(s_oh, 16)
        # tot[p, g] = sum_c oh[p, c, g]: contiguous pre-add of the column
        # halves, then a strided reduce over the remaining 32 columns
        nc.vector.tensor_tensor(
            out=scan_t[:, 0:C * G // 2], in0=oh_t[:, 0:C * G // 2],
            in1=oh_t[:, C * G // 2:C * G], op=mybir.AluOpType.add)
        nc.vector.tensor_reduce(
            out=tot_t[:],
            in_=scan_t[:, 0:C * G // 2].rearrange("p (c g) -> p g c", g=G),
            axis=mybir.AxisListType.X,
            op=mybir.AluOpType.add).then_inc(s_dve, 1)

        # Streaming per-column recurrence: with R[p, g] = a_ps[p, g] + 1
        # + sum_{c'<c} oh[p, c', g], the destination of token (p, c) is
        # dest[p, c] = sum_g oh[p, c, g] * R[p, g] (oh is one-hot over g,
        # and oh_c[g*] = 1 folds the token's own +1 into R's +1).  Four
        # tiny contiguous [128, 16] ops per column stream dest columns out
        # faster than the Pool consumes them (1.42us/col), so the first
        # scatter fires ~3us earlier than a bulk scan pipeline.  Emission
        # order keeps every same-engine RAW/WAR pair >= 2 instructions
        # apart (DVE does not interlock close hazards).
        nc.vector.wait_ge(s_pe, 1)
        r_t = scan_t[:, 0:G]
        m_t = prod_t
        nc.vector.tensor_scalar(out=r_t, in0=a_ps[:], scalar1=1.0,
                                scalar2=None, op0=mybir.AluOpType.add)

        def mult_c(c):
            nc.vector.tensor_tensor(
                out=m_t[:, c * G:(c + 1) * G],
                in0=oh_t[:, c * G:(c + 1) * G], in1=r_t,
                op=mybir.AluOpType.mult)

        def racc_c(c):
            nc.vector.tensor_tensor(
                out=r_t, in0=r_t, in1=oh_t[:, c * G:(c + 1) * G],
                op=mybir.AluOpType.add)

        def reduce_c(c):
            nc.vector.tensor_reduce(
                out=dest_f[:, c:c + 1],
                in_=m_t[:, c * G:(c + 1) * G].rearrange(
                    "p (c g) -> p c g", g=G),
                axis=mybir.AxisListType.X,
                op=mybir.AluOpType.add)

        def cast_c(c):
            # dest_i[:, c] ready => s_dve hits 2 + c
            nc.vector.tensor_copy(
                out=dest_i[:, c:c + 1],
                in_=dest_f[:, c:c + 1]).then_inc(s_dve, 1)

        def spacer(i):
            # harmless filler op to keep hazard distances >= 2 during boot
            nc.vector.tensor_copy(out=m_t[:, C * G - 1 - i:C * G - i],
                                  in_=oh_t[:, 0:1])

        # steady schedule: mult_c, cast_{c-1}, racc_c, reduce_c — every
        # RAW *and* WAR pair on r_t/m_t/dest_f lands >= 2 instructions
        # apart (the spacers cover the boot distances)
        spacer(0)
        mult_c(0)
        spacer(1)
        racc_c(0)
        reduce_c(0)
        for c in range(1, C):
            mult_c(c)
            cast_c(c - 1)
            racc_c(c)
            reduce_c(c)
        spacer(2)
        cast_c(C - 1)

        # ---------------- PE ----------------
        nc.tensor.wait_ge(s_const, 32)
        nc.tensor.wait_ge(s_dve, 1)
        nc.tensor.matmul(out=a_ps[:], lhsT=su_t, rhs=tot_t[:],
                         start=True, stop=False)
        nc.tensor.matmul(out=a_ps[:], lhsT=ones_t, rhs=cst_t,
                         start=False, stop=True).then_inc(s_pe, 1)

        # ---------------- Pool: indirect scatter-writes ----------------
        qname = ["qPoolDynamic", "qPoolDynamic1", "qPoolDynamic2",
                 "qPoolDynamic3"]
        # dummy scatter at t=0 warms the dynamic-DMA path.  The ucode only
        # supports one offset per partition and a 2D [128, D] payload per
        # call, so the main loop is one call per token column: 64 calls x
        # 128 rows of 2 KiB (~1.1 us of Pool desc-gen each, measured).
        # v4: no dma_scatter_add columns at all — the Ant calls cost
        # ~5.7 us each of serial Pool time plus a ~12 us LOAD_LIB stall in
        # front of the generic calls, and their CCE RMW re-reads the
        # output (4 MiB extra HBM traffic).  All-generic is both cheaper
        # on the Pool queue and lighter on the bus.
        nc.gpsimd.memset(dummy_idx[:], 0).then_inc(s_warm, 1)
        nc.gpsimd.memset(dummy_pay[:], 0).then_inc(s_warm, 1)
        nc.gpsimd.wait_ge(s_warm, 2)
        _indirect_scatter_write(
            nc, dummy_d[:], dummy_idx[:], dummy_pay[:],
            qname[0]).then_inc(s_sq0, 16)

        chunk_start = {c0: k for k, (c0, c1) in enumerate(CHUNKS)}

        # dest_i column c is covered by s_dve >= 2 + c (per-column
        # casts); waits are per-column early on and coarsen once the
        # index stream is ahead; each coarse wait covers every column up
        # to the next milestone.
        for c in range(C):
            if c in chunk_start:
                k = chunk_start[c]
                nc.gpsimd.wait_ge(s_xs, 16 * (k + 1))
                nc.gpsimd.wait_ge(s_xc, 16 * (k + 1))
            if c < 6:
                nc.gpsimd.wait_ge(s_dve, 2 + c)
            elif c % 8 == 6:
                nc.gpsimd.wait_ge(s_dve, 2 + min(c + 7, C - 1))
            _indirect_scatter_write(
                nc, out_d[:], dest_i[:, c:c + 1],
                xt[:, c * D:(c + 1) * D],
                qname[0]).then_inc(s_sq0, 16)
        nc.gpsimd.wait_ge(s_sq0, 16 * (1 + C))

    nc.compile()
    return nc


def _get_nc():
    global _cached
    if _cached is None:
        _cached = _build()
    return _cached


def _constants():
    cst_big = np.ascontiguousarray(
        np.triu(np.ones((P, P), np.float32), k=1))
    ones_r = np.ones((1, P), np.float32)
    cst = (np.arange(G, dtype=np.float32) * CAP - 1.0).reshape(1, G)
    cst_row = np.concatenate([ones_r, cst], axis=1)
    return cst_big, cst_row


def kernel(x, block_onehot, capacity):
    from concourse.bass_utils import run_bass_kernel_spmd

    x = np.ascontiguousarray(np.asarray(x, dtype=np.float32))
    oh = np.asarray(block_onehot, dtype=np.float32)
    if oh.ndim == 2:
        oh = np.broadcast_to(oh[None], (B,) + oh.shape)
    oh = np.ascontiguousarray(oh)
    assert x.shape == (B, N, D), x.shape
    assert oh.shape == (B, N, G), oh.shape
    assert int(capacity) == CAP, capacity
    nc = _get_nc()
    cst_big, cst_row = _constants()
    in_maps = [
        {"x": x[b], "oh": oh[b], "cst_big": cst_big, "cst_row": cst_row}
        for b in range(B)
    ]
    res = run_bass_kernel_spmd(nc, in_maps, core_ids=list(range(NCORES)))
    return np.stack([res.results[b]["out"].reshape(G, CAP, D)
                     for b in range(B)])

